# revision 11
# baseline (speedup 1.0000x reference)
"""CRF loss (forward-algorithm log-partition minus gold-path score) on 8 trn2 cores.

Strategy (data-parallel over B, 32 rows per core):
  Denominator: forward scan in probability space. With E = exp(transitions)
  as the PE stationary, each step is one matmul u = E^T @ alpha plus one DVE
  multiply alpha' = u * exp(emit_t - SHIFT). The constant SHIFT=log(128)+0.5
  cancels the expected per-step log-growth of the row-sum so fp32 stays in
  range; masking is handled by snapshotting log(row-sum) at every t >= 128
  and selecting t = len_b - 1 at the end via the mask's prefix structure
  (OH_len = maskf[t] - maskf[t+1]).
  Numerator: emission scores via one-hot matmuls accumulated over all
  (b, t-chunk) into a single PSUM tile, then a Frobenius product with I;
  transition scores from the same one-hot matmuls: PairCount = OH^T @ OH_next
  accumulated in PSUM, then a Frobenius product with the transitions table.
Output per core: scalar sum over its rows of (log_den - log_num); host
divides by B.
"""

import numpy as np
import ml_dtypes

B, T, C = 256, 512, 128
NCORES = 8
BL = B // NCORES
SHIFT = float(np.log(128.0) + 0.5)  # cancels E[log sum_j exp(em_j)] per step
TBL = C * C           # flat transitions table size
NPG = T * BL // 8     # gather pairs per 16-partition group (2048)
NHALF = NPG // 2      # per-gather indices (ISA limit ~1024 per indirect_copy)

_cache = {}


def _build_program():
    import concourse.bass as bass
    import concourse.bacc as bacc
    import concourse.tile as tile
    from concourse import mybir

    f32 = mybir.dt.float32
    bf16 = mybir.dt.bfloat16
    u16 = mybir.dt.uint16
    Alu = mybir.AluOpType
    Act = mybir.ActivationFunctionType
    Axis = mybir.AxisListType

    nc = bacc.Bacc(None)

    em_ctb = nc.dram_tensor("em_ctb", [C, T, BL], f32, kind="ExternalInput")
    em_btc = nc.dram_tensor("em_btc", [BL, T, C], f32, kind="ExternalInput")
    tagsm_tb = nc.dram_tensor("tagsm_tb", [T, BL], f32, kind="ExternalInput")
    tagsms_tb = nc.dram_tensor("tagsms_tb", [T, BL], f32, kind="ExternalInput")
    maskf_tb = nc.dram_tensor("maskf_tb", [T + 1, BL], f32, kind="ExternalInput")
    trans_in = nc.dram_tensor("trans", [C, C], f32, kind="ExternalInput")
    out_d = nc.dram_tensor("out", [1, 1], f32, kind="ExternalOutput")

    ident_in = nc.inline_tensor(np.eye(C, dtype=np.float32), name="ident")
    ones_in = nc.inline_tensor(np.ones((C, 1), np.float32), name="onescol")
    iota_in = nc.inline_tensor(
        np.tile(np.arange(C, dtype=np.float32), (C, 1)), name="iotarow"
    )

    NCH = T // 128          # 4 numerator t-chunks
    RS_K0 = 8               # rowsum chunks (16 t's each) start at t=128
    RS_K = 32               # ... through t=511

    with tile.TileContext(nc) as tc:
        with (
            tc.tile_pool(name="consts", bufs=1) as consts,
            tc.tile_pool(name="bigbuf", bufs=1) as bigbuf,
            tc.tile_pool(name="scanps", bufs=2, space="PSUM") as scanps,
            tc.tile_pool(name="accps", bufs=1, space="PSUM") as accps,
            tc.tile_pool(name="rsps", bufs=2, space="PSUM") as rsps,
            tc.tile_pool(name="oh", bufs=3) as ohpool,
            tc.tile_pool(name="emn", bufs=3) as emnpool,
            tc.tile_pool(name="logc", bufs=2) as logcpool,
            tc.tile_pool(name="dram", bufs=1, space="DRAM") as drampool,
        ):
            # ---------- constants / small inputs ----------
            trans_sb = consts.tile([C, C], f32)
            nc.sync.dma_start(out=trans_sb[:], in_=trans_in[:])
            E_sb = consts.tile([C, C], f32)
            nc.scalar.activation(out=E_sb[:], in_=trans_sb[:], func=Act.Exp)
            ident_sb = consts.tile([C, C], f32)
            nc.sync.dma_start(out=ident_sb[:], in_=ident_in[:])
            ones_sb = consts.tile([C, 1], f32)
            nc.sync.dma_start(out=ones_sb[:], in_=ones_in[:])
            iota_sb = consts.tile([C, C], f32)
            nc.sync.dma_start(out=iota_sb[:], in_=iota_in[:])
            neg_shift = consts.tile([C, 1], f32)
            nc.vector.memset(neg_shift[:], -SHIFT)

            tags_m = consts.tile([128, NCH, BL], f32)
            nc.sync.dma_start(
                out=tags_m[:],
                in_=tagsm_tb[:].rearrange("(h l) b -> l h b", l=128),
            )
            tags_ms = consts.tile([128, NCH, BL], f32)
            nc.sync.dma_start(
                out=tags_ms[:],
                in_=tagsms_tb[:].rearrange("(h l) b -> l h b", l=128),
            )
            maskf_t = consts.tile([128, NCH, BL], f32)
            nc.sync.dma_start(
                out=maskf_t[:],
                in_=maskf_tb[0:T, :].rearrange("(h l) b -> l h b", l=128),
            )
            maskf_s = consts.tile([128, NCH, BL], f32)
            nc.sync.dma_start(
                out=maskf_s[:],
                in_=maskf_tb[1 : T + 1, :].rearrange("(h l) b -> l h b", l=128),
            )

            # ---------- big buffers ----------
            exp_em = bigbuf.tile([C, T, BL], f32)
            nc.sync.dma_start(out=exp_em[:], in_=em_ctb[:])
            TCH = 64
            for k in range(T // TCH):
                nc.scalar.activation(
                    out=exp_em[:, k * TCH : (k + 1) * TCH, :],
                    in_=exp_em[:, k * TCH : (k + 1) * TCH, :],
                    func=Act.Exp, bias=neg_shift[:], scale=1.0,
                )
            S_all = bigbuf.tile([C, T, BL], f32)
            nc.vector.tensor_copy(out=S_all[:, 0, :], in_=exp_em[:, 0, :])

            # ---------- the scan ----------
            for t in range(1, T):
                u_ps = scanps.tile([C, BL], f32)
                nc.tensor.matmul(
                    u_ps[:], lhsT=E_sb[:], rhs=S_all[:, t - 1, :],
                    start=True, stop=True,
                )
                nc.vector.tensor_tensor(
                    out=S_all[:, t, :], in0=u_ps[:], in1=exp_em[:, t, :],
                    op=Alu.mult,
                )

            # ---------- row-sums + log snapshots (t >= 128) ----------
            scratch_log = drampool.tile([T * BL], f32)
            for k in range(RS_K0, RS_K):
                rs_ps = rsps.tile([1, 16 * BL], f32)
                nc.tensor.matmul(
                    rs_ps[:], lhsT=ones_sb[:, :1],
                    rhs=S_all[:, 16 * k : 16 * (k + 1), :],
                    start=True, stop=True,
                )
                logc = logcpool.tile([1, 16 * BL], f32)
                nc.scalar.activation(out=logc[:], in_=rs_ps[:], func=Act.Ln)
                nc.sync.dma_start(
                    out=scratch_log[16 * BL * k : 16 * BL * (k + 1)],
                    in_=logc[:],
                )

            # ---------- numerator: one-hot matmuls ----------
            emit_ps = accps.tile([C, C], f32)
            pair_ps = accps.tile([C, C], f32)
            for b in range(BL):
                for ch in range(NCH):
                    i = b * NCH + ch
                    em_nm = emnpool.tile([128, C], f32, tag="em_nm")
                    nc.sync.dma_start(
                        out=em_nm[:],
                        in_=em_btc[b, ch * 128 : (ch + 1) * 128, :],
                    )
                    em_bf = emnpool.tile([128, C], bf16, tag="em_bf")
                    nc.scalar.copy(out=em_bf[:], in_=em_nm[:])
                    oh = ohpool.tile([128, C], bf16, tag="oh")
                    nc.vector.tensor_tensor(
                        out=oh[:], in0=iota_sb[:],
                        in1=tags_m[:, ch, b : b + 1].to_broadcast([128, C]),
                        op=Alu.is_equal,
                    )
                    ohs = ohpool.tile([128, C], bf16, tag="ohs")
                    nc.vector.tensor_tensor(
                        out=ohs[:], in0=iota_sb[:],
                        in1=tags_ms[:, ch, b : b + 1].to_broadcast([128, C]),
                        op=Alu.is_equal,
                    )
                    nc.tensor.matmul(
                        emit_ps[:], lhsT=oh[:], rhs=em_bf[:],
                        start=(i == 0), stop=(i == BL * NCH - 1),
                        skip_group_check=True,
                    )
                    nc.tensor.matmul(
                        pair_ps[:], lhsT=oh[:], rhs=ohs[:],
                        start=(i == 0), stop=(i == BL * NCH - 1),
                        skip_group_check=True,
                    )

            # ---------- denominator combine ----------
            logRS = consts.tile([128, NCH - 1, BL], f32)
            nc.sync.dma_start(
                out=logRS[:],
                in_=scratch_log[128 * BL : T * BL].rearrange(
                    "(h l b) -> l h b", h=NCH - 1, l=128
                ),
            )
            ohl = consts.tile([128, NCH - 1, BL], f32)
            nc.vector.tensor_tensor(
                out=ohl[:], in0=maskf_t[:, 1:, :], in1=maskf_s[:, 1:, :],
                op=Alu.subtract,
            )
            den_acc = consts.tile([128, 1], f32)
            nc.vector.tensor_tensor(
                out=ohl[:], in0=ohl[:], in1=logRS[:], op=Alu.mult
            )
            nc.vector.tensor_reduce(
                out=den_acc[:], in_=ohl[:], axis=Axis.XY, op=Alu.add
            )
            L_acc = consts.tile([128, 1], f32)
            nc.vector.tensor_reduce(
                out=L_acc[:], in_=maskf_t[:], axis=Axis.XY, op=Alu.add
            )
            nc.scalar.mul(out=L_acc[:], in_=L_acc[:], mul=SHIFT)

            # ---------- numerator frobenius ----------
            emit_acc = consts.tile([128, 1], f32)
            nc.vector.tensor_tensor(
                out=emit_ps[:], in0=emit_ps[:], in1=ident_sb[:], op=Alu.mult
            )
            nc.vector.tensor_reduce(
                out=emit_acc[:], in_=emit_ps[:], axis=Axis.X, op=Alu.add
            )
            pair_acc = consts.tile([128, 1], f32)
            nc.vector.tensor_tensor(
                out=pair_ps[:], in0=pair_ps[:], in1=trans_sb[:], op=Alu.mult
            )
            nc.vector.tensor_reduce(
                out=pair_acc[:], in_=pair_ps[:], axis=Axis.X, op=Alu.add
            )

            # ---------- final reduce to scalar ----------
            fin = consts.tile([128, 1], f32)
            nc.vector.tensor_tensor(
                out=fin[:], in0=den_acc[:], in1=L_acc[:], op=Alu.add
            )
            nc.vector.tensor_tensor(
                out=fin[:], in0=fin[:], in1=emit_acc[:], op=Alu.subtract
            )
            nc.vector.tensor_tensor(
                out=fin[:], in0=fin[:], in1=pair_acc[:], op=Alu.subtract
            )
            fin_ps = rsps.tile([1, 1], f32, tag="fin")
            nc.tensor.matmul(
                fin_ps[:], lhsT=ones_sb[:, :1], rhs=fin[:],
                start=True, stop=True,
            )
            res_sb = consts.tile([1, 1], f32)
            nc.scalar.copy(out=res_sb[:], in_=fin_ps[:])
            nc.sync.dma_start(out=out_d[:], in_=res_sb[:])

    nc.compile()
    return nc


def _prep_inputs(emissions, tags, mask, transitions):
    em = np.ascontiguousarray(np.asarray(emissions), dtype=np.float32)
    tg = np.asarray(tags).astype(np.int32)
    mk = np.asarray(mask).astype(bool)
    tr = np.ascontiguousarray(np.asarray(transitions), dtype=np.float32)


    in_maps = []
    for core in range(NCORES):
        b0, b1 = core * BL, (core + 1) * BL
        em_c = em[b0:b1]
        tg_c = tg[b0:b1].T                            # [T, BL] int32
        mk_c = mk[b0:b1].T.astype(np.float32)         # [T, BL]
        pad_f = np.zeros((1, BL), np.float32)

        # masked tags (+1000 where mask off) for the one-hot builds
        tags_m = (tg_c + 1000.0 * (1.0 - mk_c)).astype(np.float32)
        tg_next = np.vstack([tg_c[1:], np.zeros((1, BL), np.int32)])
        mk_next = np.vstack([mk_c[1:], pad_f])
        tags_ms = (tg_next + 1000.0 * (1.0 - mk_next)).astype(np.float32)

        in_maps.append({
            "em_ctb": np.ascontiguousarray(em_c.transpose(2, 1, 0)),
            "em_btc": np.ascontiguousarray(em_c),
            "tagsm_tb": np.ascontiguousarray(tags_m),
            "tagsms_tb": np.ascontiguousarray(tags_ms),
            "maskf_tb": np.ascontiguousarray(np.vstack([mk_c, pad_f])),
            "trans": tr,
        })
    return in_maps


def kernel(emissions, tags, mask, transitions, _want_results=False, **_run_kw):
    from concourse.bass_utils import run_bass_kernel_spmd

    if "nc" not in _cache:
        _cache["nc"] = _build_program()
    nc = _cache["nc"]

    in_maps = _prep_inputs(emissions, tags, mask, transitions)
    res = run_bass_kernel_spmd(nc, in_maps, core_ids=list(range(NCORES)), **_run_kw)
    total = sum(float(r["out"][0, 0]) for r in res.results)
    out = np.float32(total / B)
    if _want_results:
        return out, res
    return out
